# revision 1
# baseline (speedup 1.0000x reference)
"""EmbeddingBag-mean (padded ragged gather + masked mean) on 8 Trainium2 cores.

Strategy (data-parallel over batch, per the sharding hint):
  - Each of the 8 cores owns B/8 = 2048 samples; the embedding table is
    replicated to every core's HBM (augmented with one zero row at index V).
  - Host prep: indices -> int32; within each core, samples are sorted by
    descending length so each block of 128 samples only needs G_b =
    max-length-in-block gather slots; padded slots point at the zero row.
  - Device kernel (per core), per block of 128 samples:
      1. G_b indirect DMA gathers (one index per partition per slot):
         g[p, l, :] = table[idx[p, l], :]
      2. one DVE tensor_reduce over slots (strided AP view [P, D, G_b])
      3. ACT Copy-with-scale by 1/max(len,1) (per-partition scalar)
      4. DMA the [128, 64] block out
  - Host un-permutes (inverse of the length sort) and concatenates the
    8 per-core outputs.

The per-block slot counts G_b depend on the input lengths, so the Bass
module is built per distinct slot schedule (cached).
"""

import numpy as np

try:
    import concourse.bacc as bacc
except ImportError:  # harness containers keep the repo at /opt/trn_rl_repo
    import sys

    sys.path.insert(0, "/opt/trn_rl_repo")
    import concourse.bacc as bacc

import concourse.bass as bass
import concourse.mybir as mybir
import concourse.tile as tile
from concourse import bass_utils

B, L, V, D = 16384, 50, 100000, 64
NCORES = 8
P = 128
BC = B // NCORES  # 2048 samples per core
NBLK = BC // P  # 16 blocks of 128 samples

_CACHE: dict = {}


def build(g_list, reps: int = 1, gbufs: int = 4):
    """Build + bacc-compile the per-core Bass module.

    g_list: per-block gather slot counts (len NBLK, each in [1, L]).
    reps > 1 wraps the block loop in tc.For_i (same outputs each
    iteration) -- used only for wall-clock slope timing in test.py.
    """
    g_list = list(g_list)
    assert len(g_list) == NBLK and all(1 <= g <= L for g in g_list)
    W = sum(g_list)
    offs = np.cumsum([0] + g_list).tolist()
    g_max = max(g_list)

    nc = bacc.Bacc("TRN2", target_bir_lowering=False, debug=False)
    table = nc.dram_tensor("table", [V + 1, D], mybir.dt.float32, kind="ExternalInput")
    idx = nc.dram_tensor("idx", [P, W], mybir.dt.int32, kind="ExternalInput")
    inv_len = nc.dram_tensor("inv_len", [P, NBLK], mybir.dt.float32, kind="ExternalInput")
    out = nc.dram_tensor("out", [NBLK, P, D], mybir.dt.float32, kind="ExternalOutput")

    with tile.TileContext(nc) as tc:
        with (
            tc.tile_pool(name="const", bufs=1) as cpool,
            tc.tile_pool(name="gather", bufs=gbufs) as gpool,
            tc.tile_pool(name="res", bufs=4) as rpool,
        ):
            idx_sb = cpool.tile([P, W], mybir.dt.int32)
            nc.sync.dma_start(idx_sb[:], idx.ap())
            invl_sb = cpool.tile([P, NBLK], mybir.dt.float32)
            nc.sync.dma_start(invl_sb[:], inv_len.ap())

            def body():
                for b in range(NBLK):
                    gb = g_list[b]
                    g = gpool.tile([P, g_max, D], mybir.dt.float32, tag="g")
                    gflat = g[:].rearrange("p l d -> p (l d)")
                    for l in range(gb):
                        nc.gpsimd.indirect_dma_start(
                            out=gflat[:, l * D : (l + 1) * D],
                            out_offset=None,
                            in_=table.ap(),
                            in_offset=bass.IndirectOffsetOnAxis(
                                ap=idx_sb[:, offs[b] + l : offs[b] + l + 1], axis=0
                            ),
                        )
                    red = rpool.tile([P, D], mybir.dt.float32, tag="red")
                    nc.vector.tensor_reduce(
                        out=red[:],
                        in_=g[:, :gb, :].rearrange("p l d -> p d l"),
                        axis=mybir.AxisListType.X,
                        op=mybir.AluOpType.add,
                    )
                    o = rpool.tile([P, D], mybir.dt.float32, tag="o")
                    nc.scalar.activation(
                        o[:],
                        red[:],
                        mybir.ActivationFunctionType.Copy,
                        scale=invl_sb[:, b : b + 1],
                    )
                    nc.sync.dma_start(out.ap()[b], o[:])

            if reps == 1:
                body()
            else:
                with tc.For_i(0, reps, 1):
                    body()

    nc.compile()
    return nc


def preprocess(table, indices, lengths):
    """Host prep. Returns (in_maps, g_list, perms) where perms[c] maps
    device row order (sorted) back to original sample order."""
    table = np.ascontiguousarray(np.asarray(table, dtype=np.float32))
    table_aug = np.concatenate([table, np.zeros((1, D), np.float32)], axis=0)

    idx32 = np.asarray(indices).astype(np.int32)  # [B, L]
    lens = np.asarray(lengths).astype(np.int64)  # [B]
    valid = np.arange(L, dtype=np.int64)[None, :] < lens[:, None]
    idx32 = np.where(valid, idx32, np.int32(V))
    inv_len = (1.0 / np.maximum(lens, 1)).astype(np.float32)  # [B]

    # Sort each core's samples by descending length; block b then only
    # needs G_b = lens_sorted[128*b] gather slots. g_list must be shared
    # across cores (one compiled module), so take the per-block max.
    perms, g_lists = [], []
    for c in range(NCORES):
        s = slice(c * BC, (c + 1) * BC)
        perm = np.argsort(-lens[s], kind="stable")
        perms.append(perm)
        ls = lens[s][perm]
        g_lists.append(np.maximum(ls[::P][:NBLK], 1))
    g_list = np.maximum.reduce(g_lists).astype(int).tolist()
    W = int(np.sum(g_list))
    offs = np.cumsum([0] + g_list)

    in_maps = []
    for c in range(NCORES):
        s = slice(c * BC, (c + 1) * BC)
        idx_c = idx32[s][perms[c]]  # [BC, L] sorted by desc length
        invl_c = inv_len[s][perms[c]]
        idx_dev = np.full((P, W), V, np.int32)
        for b in range(NBLK):
            gb = g_list[b]
            blk = idx_c[b * P : (b + 1) * P, :gb]  # [P, gb]
            idx_dev[:, offs[b] : offs[b] + gb] = blk
        invl_dev = invl_c.reshape(NBLK, P).T  # [P, NBLK]
        in_maps.append(
            {
                "table": table_aug,
                "idx": np.ascontiguousarray(idx_dev),
                "inv_len": np.ascontiguousarray(invl_dev),
            }
        )
    return in_maps, g_list, perms


def kernel(table, indices, lengths):
    in_maps, g_list, perms = preprocess(table, indices, lengths)
    key = tuple(g_list)
    nc = _CACHE.get(key)
    if nc is None:
        nc = _CACHE[key] = build(g_list)
    res = bass_utils.run_bass_kernel_spmd(nc, in_maps, core_ids=list(range(NCORES)))
    full = np.empty((B, D), np.float32)
    for c in range(NCORES):
        rows = res.results[c]["out"].reshape(BC, D)
        full[c * BC : (c + 1) * BC][perms[c]] = rows
    return full



# revision 4
# speedup vs baseline: 1.1910x; 1.1910x over previous
"""EmbeddingBag-mean (padded ragged gather + masked mean) on 8 Trainium2 cores.

Strategy (data-parallel over batch):
  - Each of the 8 cores owns B/8 = 2048 samples; the embedding table is
    replicated to every core's HBM as fp16, rows padded to 128 elements
    (256 B stride) so the MoE `dma_gather` ucode (stride in 256 B units,
    int16 indices) can gather single 128 B rows.
  - int16 indices only reach 32768 rows, so the gather runs as 4 passes
    over overlapping 32768-row windows (bases ~22412 apart).  A zero
    sentinel row sits at each window base (relative index 0).  Because the
    windows overlap, each sample distributes its indices among feasible
    passes to equalize its per-pass counts, which keeps the per-block
    per-pass slot maxima near len/4 (instead of len/4 + 3 sigma).
  - Samples are globally length-sorted and dealt to (block, core,
    partition) so each block of 128 partitions holds samples of nearly
    equal length; block b needs G[b][q] gather slots for pass q (max over
    the 8 cores' blocks; one SPMD module).  Pad slots -> sentinel row.
  - Device kernel (per core), per block of 128 samples:
      1. 4x dma_gather (one per pass, 4 SWDGE queues round-robin):
         g[p, off_q + j, :] = window_q[idx16[...], :64]   (128 B descs)
      2. one DVE tensor_reduce over all slot columns (fp16 in, fp32 out)
      3. ACT Copy-with-scale by 1/max(len,1) (per-partition scalar)
      4. DMA the [128, 64] fp32 block out
  - Host un-permutes the global deal and returns [B, 64] fp32.
"""

import numpy as np

try:
    import concourse.bacc as bacc
except ImportError:  # harness containers keep the repo at /opt/trn_rl_repo
    import sys

    sys.path.insert(0, "/opt/trn_rl_repo")
    import concourse.bacc as bacc

import concourse.bass as bass
import concourse.mybir as mybir
import concourse.tile as tile
from concourse import bass_utils

B, L, V, D = 16384, 50, 100000, 64
NCORES = 8
P = 128
BC = B // NCORES  # 2048 samples per core
NBLK = BC // P  # 16 blocks of 128 samples
NQ = 4  # gather passes (overlapping windows)
WIN = 32768  # int16-reachable rows per pass
DEVROWS = V + NQ  # table + one zero sentinel per window

# window bases in device-table row space (sentinel zero row at each base)
_SPACING = -(-(DEVROWS - WIN) // (NQ - 1))  # ceil
BASES = [q * _SPACING for q in range(NQ)]
assert BASES[-1] + WIN >= DEVROWS

_CACHE: dict = {}


def _manual_dma_gather(nc, out_ap, in_ap, idxs_ap, num_idxs, elem_size,
                       queue_num, single_packet):
    """dma_gather without the elem_size_bytes%256 assert (stride is 256B)."""
    gp = nc.gpsimd
    _in_ap = gp.lower_ap_dma(in_ap, for_custom_bir_dma=True)
    _idxs_ap = gp.lower_ap(idxs_ap)
    _out_ap = gp.lower_ap(out_ap)
    stride_bytes = in_ap.ap[0][0] * mybir.dt.size(in_ap.dtype)
    assert stride_bytes % 256 == 0
    return gp.add_instruction(
        mybir.InstDMAGatherAnt(
            name=nc.get_next_instruction_name(),
            ins=[*_in_ap, _idxs_ap, gp.lower_val_access(gp.to_reg(num_idxs))],
            outs=[_out_ap],
            transpose=False,
            num_idxs=num_idxs,
            elem_size=elem_size,
            stride_bytes_256=stride_bytes // 256,
            gen_mode=0,
            single_packet=single_packet,
            queue_num=queue_num,
            sbuf_tokens_per_rank=0,
            sbuf_free_dim_per_rank=0,
            sbuf_free_dim_pad_per_rank=0,
            sbuf_byte_offset=0,
        )
    )


def build(g_sched, reps: int = 1):
    """Build + compile the per-core Bass module.

    g_sched: [NBLK][NQ] gather slot counts (>=1 each).
    reps > 1 wraps the block loop in tc.For_i for slope timing.
    """
    g_sched = [list(r) for r in g_sched]
    assert len(g_sched) == NBLK and all(len(r) == NQ for r in g_sched)
    gtot = [sum(r) for r in g_sched]
    g_max = max(gtot)
    # idx16 column layout: per (block, pass) a run of G*P/16 int16 columns
    wcols = [[g * P // 16 for g in r] for r in g_sched]
    WC = sum(sum(r) for r in wcols)

    nc = bacc.Bacc("TRN2", target_bir_lowering=False, debug=False,
                   num_swdge_queues=NQ)
    table = nc.dram_tensor("table", [DEVROWS, P], mybir.dt.float16,
                           kind="ExternalInput")
    idx = nc.dram_tensor("idx", [P, WC], mybir.dt.int16, kind="ExternalInput")
    inv_len = nc.dram_tensor("inv_len", [P, NBLK], mybir.dt.float32,
                             kind="ExternalInput")
    out = nc.dram_tensor("out", [NBLK, P, D], mybir.dt.float32,
                         kind="ExternalOutput")

    with tile.TileContext(nc) as tc:
        with (
            tc.tile_pool(name="const", bufs=1) as cpool,
            tc.tile_pool(name="gather", bufs=4) as gpool,
            tc.tile_pool(name="res", bufs=4) as rpool,
        ):
            idx_sb = cpool.tile([P, WC], mybir.dt.int16)
            nc.sync.dma_start(idx_sb[:], idx.ap())
            invl_sb = cpool.tile([P, NBLK], mybir.dt.float32)
            nc.sync.dma_start(invl_sb[:], inv_len.ap())

            def body():
                col = 0
                for b in range(NBLK):
                    g = gpool.tile([P, g_max, D], mybir.dt.float16, tag="g")
                    off = 0
                    for q in range(NQ):
                        gq = g_sched[b][q]
                        nidx = gq * P
                        win = table.ap()[BASES[q] : BASES[q] + WIN, :D]
                        _manual_dma_gather(
                            nc,
                            g[:, off : off + gq, :],
                            win,
                            idx_sb[:, col : col + wcols[b][q]],
                            nidx,
                            D,
                            queue_num=q,
                            single_packet=(gq <= 8),
                        )
                        off += gq
                        col += wcols[b][q]
                    red = rpool.tile([P, D], mybir.dt.float32, tag="red")
                    nc.vector.tensor_reduce(
                        out=red[:],
                        in_=g[:, : gtot[b], :].rearrange("p l d -> p d l"),
                        axis=mybir.AxisListType.X,
                        op=mybir.AluOpType.add,
                    )
                    o = rpool.tile([P, D], mybir.dt.float32, tag="o")
                    nc.scalar.activation(
                        o[:],
                        red[:],
                        mybir.ActivationFunctionType.Copy,
                        scale=invl_sb[:, b : b + 1],
                    )
                    nc.sync.dma_start(out.ap()[b], o[:])

            if reps == 1:
                body()
            else:
                with tc.For_i(0, reps, 1):
                    body()

    nc.compile()
    return nc


def _dev_table(table):
    """fp16 device table [DEVROWS, 128]: zero sentinel at each window base,
    original row r at device position devpos[r]."""
    t16 = np.asarray(table, dtype=np.float32).astype(np.float16)
    dev = np.zeros((DEVROWS, P), np.float16)
    devpos = np.empty(V, np.int64)
    src = 0
    for pos in range(DEVROWS):
        if pos in BASES:
            continue  # zero sentinel
        dev[pos, :D] = t16[src]
        devpos[src] = pos
        src += 1
    assert src == V
    return dev, devpos


def _balance_passes(devrows_sample):
    """Assign each device-row index to a feasible pass, equalizing per-pass
    counts. Returns list of NQ lists of window-relative indices."""
    groups = [[] for _ in range(NQ)]
    flex = []
    for d in devrows_sample:
        feas = [q for q in range(NQ) if BASES[q] <= d < BASES[q] + WIN]
        if len(feas) == 1:
            groups[feas[0]].append(d - BASES[feas[0]])
        else:
            flex.append((d, feas))
    for d, feas in flex:
        q = min(feas, key=lambda q: len(groups[q]))
        groups[q].append(d - BASES[q])
    return groups


def preprocess(table, indices, lengths):
    """Host prep. Returns (in_maps, g_sched, order) where order[r] is the
    original sample id at global dealt rank r."""
    dev, devpos = _dev_table(table)

    idx_np = np.asarray(indices, dtype=np.int64)  # [B, L]
    lens = np.asarray(lengths).astype(np.int64)  # [B]
    inv_len = (1.0 / np.maximum(lens, 1)).astype(np.float32)

    # global length sort; rank r -> (block r//1024, core (r%1024)//128,
    # partition r%128)
    order = np.argsort(-lens, kind="stable")

    # per-sample pass groups (window-relative indices)
    sample_groups = []
    for s in range(B):
        drows = devpos[idx_np[s, : lens[s]]]
        sample_groups.append(_balance_passes(drows))

    # shared slot schedule: max over the 1024 samples of each global block
    g_sched = []
    for b in range(NBLK):
        ranks = order[b * 1024 : (b + 1) * 1024]
        g = [1] * NQ
        for s in ranks:
            for q in range(NQ):
                g[q] = max(g[q], len(sample_groups[s][q]))
        g_sched.append(g)

    wcols = [[g * P // 16 for g in r] for r in g_sched]
    WC = sum(sum(r) for r in wcols)

    in_maps = []
    for c in range(NCORES):
        idx16 = np.zeros((P, WC), np.int16)
        invl_dev = np.empty((P, NBLK), np.float32)
        col = 0
        for b in range(NBLK):
            ranks = order[b * 1024 + c * P : b * 1024 + (c + 1) * P]
            invl_dev[:, b] = inv_len[ranks]
            for q in range(NQ):
                gq = g_sched[b][q]
                blk = np.zeros((P, gq), np.int16)  # sentinel rel idx 0
                for p, s in enumerate(ranks):
                    grp = sample_groups[s][q]
                    blk[p, : len(grp)] = grp
                # stream order i = c*128 + p -> wrap int16 [16, nidx/16] x8
                flat = blk.T.ravel()  # [gq*128]
                w = flat.reshape(gq * P // 16, 16).T  # [16, cols]
                nw = wcols[b][q]
                idx16[:, col : col + nw] = np.tile(w, (8, 1))
                col += nw
        in_maps.append(
            {
                "table": dev,
                "idx": np.ascontiguousarray(idx16),
                "inv_len": np.ascontiguousarray(invl_dev),
            }
        )
    return in_maps, g_sched, order


def kernel(table, indices, lengths):
    in_maps, g_sched, order = preprocess(table, indices, lengths)
    key = tuple(tuple(r) for r in g_sched)
    nc = _CACHE.get(key)
    if nc is None:
        nc = _CACHE[key] = build(g_sched)
    res = bass_utils.run_bass_kernel_spmd(nc, in_maps, core_ids=list(range(NCORES)))
    full = np.empty((B, D), np.float32)
    for b in range(NBLK):
        for c in range(NCORES):
            ranks = order[b * 1024 + c * P : b * 1024 + (c + 1) * P]
            full[ranks] = res.results[c]["out"][b]
    return full


# revision 13
# speedup vs baseline: 2.1985x; 1.8458x over previous
"""EmbeddingBag-mean (padded ragged gather + masked mean) on 8 Trainium2 cores.

Strategy (data-parallel over batch):
  - Each of the 8 cores owns B/8 = 2048 samples; the embedding table is
    replicated to every core's HBM as fp16, rows padded to 128 elements
    (256 B stride) so the MoE `dma_gather` ucode (stride in 256 B units,
    int16 indices) can gather single 128 B rows.
  - int16 indices only reach 32768 rows, so the gather runs as 4 passes
    over overlapping 32768-row windows (bases ~22412 apart).  A zero
    sentinel row sits at each window base (relative index 0).  Because the
    windows overlap, each sample distributes its indices among feasible
    passes to equalize its per-pass counts, which keeps the per-block
    per-pass slot maxima near len/4 (instead of len/4 + 3 sigma).
  - Samples are globally length-sorted and dealt to (block, core,
    partition) so each block of 128 partitions holds samples of nearly
    equal length; block b needs G[b][q] gather slots for pass q (max over
    the 8 cores' blocks; one SPMD module).  Pad slots -> sentinel row.
  - Device kernel (per core), per block of 128 samples:
      1. 4x dma_gather (one per pass, 4 SWDGE queues round-robin):
         g[p, off_q + j, :] = window_q[idx16[...], :64]   (128 B descs)
      2. one DVE tensor_reduce over all slot columns (fp16 in, fp32 out)
      3. ACT Copy-with-scale by 1/max(len,1) (per-partition scalar)
      4. DMA the [128, 64] fp32 block out
  - Host un-permutes the global deal and returns [B, 64] fp32.
"""

import numpy as np

try:
    import concourse.bacc as bacc
except ImportError:  # harness containers keep the repo at /opt/trn_rl_repo
    import sys

    sys.path.insert(0, "/opt/trn_rl_repo")
    import concourse.bacc as bacc

import concourse.bass as bass
import concourse.mybir as mybir
import concourse.tile as tile
from concourse import bass_utils

B, L, V, D = 16384, 50, 100000, 64
NCORES = 8
P = 128
BC = B // NCORES  # 2048 samples per core
NBLK = BC // P  # 16 blocks of 128 samples
NQ = 4  # gather passes (overlapping windows)
WIN = 32768  # int16-reachable rows per pass
DEVROWS = V + NQ  # table + one zero sentinel per window

# window bases in device-table row space (sentinel zero row at each base)
_SPACING = -(-(DEVROWS - WIN) // (NQ - 1))  # ceil
BASES = [q * _SPACING for q in range(NQ)]
assert BASES[-1] + WIN >= DEVROWS

_CACHE: dict = {}


def _manual_dma_gather(nc, out_ap, in_ap, idxs_ap, num_idxs, elem_size,
                       queue_num, single_packet):
    """dma_gather without the elem_size_bytes%256 assert (stride is 256B)."""
    gp = nc.gpsimd
    _in_ap = gp.lower_ap_dma(in_ap, for_custom_bir_dma=True)
    _idxs_ap = gp.lower_ap(idxs_ap)
    _out_ap = gp.lower_ap(out_ap)
    stride_bytes = in_ap.ap[0][0] * mybir.dt.size(in_ap.dtype)
    assert stride_bytes % 256 == 0
    return gp.add_instruction(
        mybir.InstDMAGatherAnt(
            name=nc.get_next_instruction_name(),
            ins=[*_in_ap, _idxs_ap, gp.lower_val_access(gp.to_reg(num_idxs))],
            outs=[_out_ap],
            transpose=False,
            num_idxs=num_idxs,
            elem_size=elem_size,
            stride_bytes_256=stride_bytes // 256,
            gen_mode=0,
            single_packet=single_packet,
            queue_num=queue_num,
            sbuf_tokens_per_rank=0,
            sbuf_free_dim_per_rank=0,
            sbuf_free_dim_pad_per_rank=0,
            sbuf_byte_offset=0,
        )
    )


def build(g_sched, reps: int = 1, mode: str = "full", qpat=None):
    """Build + compile the per-core Bass module.

    g_sched: [NBLK][NQ] gather slot counts (>=1 each).
    reps > 1 wraps the block loop in tc.For_i for slope timing.
    mode: "full" | "gather" (skip reduce/scale/out) | "nored" (skip reduce).
    """
    g_sched = [list(r) for r in g_sched]
    assert len(g_sched) == NBLK and all(len(r) == NQ for r in g_sched)
    gtot = [sum(r) for r in g_sched]
    g_max = max(gtot)
    # idx16 column layout: per (block, pass) a run of G*P/16 int16 columns
    wcols = [[g * P // 16 for g in r] for r in g_sched]
    WC = sum(sum(r) for r in wcols)

    nc = bacc.Bacc("TRN2", target_bir_lowering=False, debug=False,
                   num_swdge_queues=NQ)
    table = nc.dram_tensor("table", [DEVROWS, P], mybir.dt.float16,
                           kind="ExternalInput")
    idx = nc.dram_tensor("idx", [P, WC], mybir.dt.int16, kind="ExternalInput")
    inv_len = nc.dram_tensor("inv_len", [P, NBLK], mybir.dt.float32,
                             kind="ExternalInput")
    out = nc.dram_tensor("out", [NBLK, P, D], mybir.dt.float32,
                         kind="ExternalOutput")

    with tile.TileContext(nc) as tc:
        with (
            tc.tile_pool(name="const", bufs=1) as cpool,
            tc.tile_pool(name="gather", bufs=4) as gpool,
            tc.tile_pool(name="res", bufs=4) as rpool,
        ):
            idx_sb = cpool.tile([P, WC], mybir.dt.int16)
            nc.sync.dma_start(idx_sb[:], idx.ap())
            invl_sb = cpool.tile([P, NBLK], mybir.dt.float32)
            nc.sync.dma_start(invl_sb[:], inv_len.ap())

            # one gather per (block, pass).  Queue pattern alternates between
            # blocks so the big outer passes split evenly across queues, while
            # staying periodic in issue order (Tile's DMASW lanes lock to the
            # queue of their first user, so the 8-long pattern must repeat).
            QPAT = qpat or [[0, 1, 2, 3], [2, 3, 0, 1]]

            def body():
                col = 0
                for b in range(NBLK):
                    g = gpool.tile([P, g_max, D], mybir.dt.float16, tag="g")
                    off = 0
                    for q in range(NQ):
                        gq = g_sched[b][q]
                        win = table.ap()[BASES[q] : BASES[q] + WIN, :D]
                        _manual_dma_gather(
                            nc,
                            g[:, off : off + gq, :],
                            win,
                            idx_sb[:, col : col + wcols[b][q]],
                            gq * P,
                            D,
                            queue_num=QPAT[b % 2][q],
                            single_packet=(gq <= 8),
                        )
                        off += gq
                        col += wcols[b][q]
                    if mode == "gather":
                        continue
                    red = rpool.tile([P, D], mybir.dt.float32, tag="red")
                    if mode == "nored":
                        nc.vector.tensor_copy(red[:], g[:, 0, :])
                    else:
                        nc.vector.tensor_reduce(
                            out=red[:],
                            in_=g[:, : gtot[b], :].rearrange("p l d -> p d l"),
                            axis=mybir.AxisListType.X,
                            op=mybir.AluOpType.add,
                        )
                    o = rpool.tile([P, D], mybir.dt.float32, tag="o")
                    nc.scalar.activation(
                        o[:],
                        red[:],
                        mybir.ActivationFunctionType.Copy,
                        scale=invl_sb[:, b : b + 1],
                    )
                    nc.sync.dma_start(out.ap()[b], o[:])

            if reps == 1:
                body()
            else:
                with tc.For_i(0, reps, 1):
                    body()

    nc.compile()
    return nc


def _dev_table(table):
    """fp16 device table [DEVROWS, 128]: zero sentinel at each window base,
    original row r at device position devpos[r]."""
    t16 = np.asarray(table, dtype=np.float32).astype(np.float16)
    dev = np.zeros((DEVROWS, P), np.float16)
    devpos = np.empty(V, np.int64)
    src = 0
    for pos in range(DEVROWS):
        if pos in BASES:
            continue  # zero sentinel
        dev[pos, :D] = t16[src]
        devpos[src] = pos
        src += 1
    assert src == V
    return dev, devpos


def _balance_passes(devrows_sample):
    """Assign each device-row index to a feasible pass, equalizing per-pass
    counts. Returns list of NQ lists of window-relative indices."""
    groups = [[] for _ in range(NQ)]
    flex = []
    for d in devrows_sample:
        feas = [q for q in range(NQ) if BASES[q] <= d < BASES[q] + WIN]
        if len(feas) == 1:
            groups[feas[0]].append(d - BASES[feas[0]])
        else:
            flex.append((d, feas))
    for d, feas in flex:
        q = min(feas, key=lambda q: len(groups[q]))
        groups[q].append(d - BASES[q])
    return groups


def preprocess(table, indices, lengths):
    """Host prep. Returns (in_maps, g_sched, order) where order[r] is the
    original sample id at global dealt rank r."""
    dev, devpos = _dev_table(table)

    idx_np = np.asarray(indices, dtype=np.int64)  # [B, L]
    lens = np.asarray(lengths).astype(np.int64)  # [B]
    inv_len = (1.0 / np.maximum(lens, 1)).astype(np.float32)

    # per-sample pass groups (window-relative indices)
    sample_groups = []
    cnt = np.zeros((B, NQ), np.int64)
    for s in range(B):
        drows = devpos[idx_np[s, : lens[s]]]
        sample_groups.append(_balance_passes(drows))
        cnt[s] = [len(g) for g in sample_groups[s]]

    # greedy deal: assign samples to the 16 global blocks (1024 each) to
    # minimize the per-block per-pass maxima; rank r -> (block r//1024,
    # core (r%1024)//128, partition r%128)
    key = cnt.max(1) * 64 + lens
    pool = np.argsort(-key, kind="stable")
    gmax = np.zeros((NBLK, NQ), np.int64)
    fill = np.zeros(NBLK, np.int64)
    assign = np.empty(B, np.int64)
    for s in pool:
        best, bc = -1, None
        for b in range(NBLK):
            if fill[b] >= 1024:
                continue
            cost = np.maximum(gmax[b], cnt[s]).sum() - gmax[b].sum()
            if bc is None or cost < bc:
                best, bc = b, cost
        assign[s] = best
        gmax[best] = np.maximum(gmax[best], cnt[s])
        fill[best] += 1
    order = np.concatenate([pool[assign[pool] == b] for b in range(NBLK)])

    g_sched = [[int(x) for x in np.maximum(gmax[b], 1)] for b in range(NBLK)]

    wcols = [[g * P // 16 for g in r] for r in g_sched]
    WC = sum(sum(r) for r in wcols)

    in_maps = []
    for c in range(NCORES):
        idx16 = np.zeros((P, WC), np.int16)
        invl_dev = np.empty((P, NBLK), np.float32)
        col = 0
        for b in range(NBLK):
            ranks = order[b * 1024 + c * P : b * 1024 + (c + 1) * P]
            invl_dev[:, b] = inv_len[ranks]
            for q in range(NQ):
                gq = g_sched[b][q]
                blk = np.zeros((P, gq), np.int16)  # sentinel rel idx 0
                for p, s in enumerate(ranks):
                    grp = sample_groups[s][q]
                    blk[p, : len(grp)] = grp
                # stream order i = c*128 + p -> wrap int16 [16, nidx/16] x8
                flat = blk.T.ravel()  # [gq*128]
                w = flat.reshape(gq * P // 16, 16).T  # [16, cols]
                nw = wcols[b][q]
                idx16[:, col : col + nw] = np.tile(w, (8, 1))
                col += nw
        in_maps.append(
            {
                "table": dev,
                "idx": np.ascontiguousarray(idx16),
                "inv_len": np.ascontiguousarray(invl_dev),
            }
        )
    return in_maps, g_sched, order


def kernel(table, indices, lengths):
    in_maps, g_sched, order = preprocess(table, indices, lengths)
    key = tuple(tuple(r) for r in g_sched)
    nc = _CACHE.get(key)
    if nc is None:
        nc = _CACHE[key] = build(g_sched)
    res = bass_utils.run_bass_kernel_spmd(nc, in_maps, core_ids=list(range(NCORES)))
    full = np.empty((B, D), np.float32)
    for b in range(NBLK):
        for c in range(NCORES):
            ranks = order[b * 1024 + c * P : b * 1024 + (c + 1) * P]
            full[ranks] = res.results[c]["out"][b]
    return full


# revision 16
# speedup vs baseline: 2.7879x; 1.2681x over previous
"""EmbeddingBag-mean (padded ragged gather + masked mean) on 8 Trainium2 cores.

Strategy (data-parallel over batch):
  - Each of the 8 cores owns B/8 = 2048 samples; the embedding table is
    replicated to every core's HBM as fp16, rows padded to 128 elements
    (256 B stride) so the MoE `dma_gather` ucode (stride in 256 B units,
    int16 indices) can gather single 128 B rows.
  - int16 indices only reach 32768 rows, so the gather runs as 4 passes
    over overlapping 32768-row windows (bases ~22412 apart).  A zero
    sentinel row sits at each window base (relative index 0).  Because the
    windows overlap, each sample distributes its indices among feasible
    passes to equalize its per-pass counts, which keeps the per-block
    per-pass slot maxima near len/4 (instead of len/4 + 3 sigma).
  - Samples are globally length-sorted and dealt to (block, core,
    partition) so each block of 128 partitions holds samples of nearly
    equal length; block b needs G[b][q] gather slots for pass q (max over
    the 8 cores' blocks; one SPMD module).  Pad slots -> sentinel row.
  - Device kernel (per core), per block of 128 samples:
      1. 4x dma_gather (one per pass, 4 SWDGE queues round-robin):
         g[p, off_q + j, :] = window_q[idx16[...], :64]   (128 B descs)
      2. one DVE tensor_reduce over all slot columns (fp16 in, fp32 out)
      3. ACT Copy-with-scale by 1/max(len,1) (per-partition scalar)
      4. DMA the [128, 64] fp32 block out
  - Host un-permutes the global deal and returns [B, 64] fp32.
"""

import numpy as np

try:
    import concourse.bacc as bacc
except ImportError:  # harness containers keep the repo at /opt/trn_rl_repo
    import sys

    sys.path.insert(0, "/opt/trn_rl_repo")
    import concourse.bacc as bacc

import concourse.bass as bass
import concourse.mybir as mybir
import concourse.tile as tile
from concourse import bass_utils

B, L, V, D = 16384, 50, 100000, 64
NCORES = 8
P = 128
BC = B // NCORES  # 2048 samples per core
NBLK = BC // P  # 16 blocks of 128 samples
NQ = 4  # gather passes (overlapping windows)
WIN = 32768  # int16-reachable rows per pass
DEVROWS = V + NQ  # table + one zero sentinel per window

# window bases in device-table row space (sentinel zero row at each base)
_SPACING = -(-(DEVROWS - WIN) // (NQ - 1))  # ceil
BASES = [q * _SPACING for q in range(NQ)]
assert BASES[-1] + WIN >= DEVROWS

_CACHE: dict = {}


def _manual_dma_gather(nc, out_ap, in_ap, idxs_ap, num_idxs, elem_size,
                       queue_num, single_packet):
    """dma_gather without the elem_size_bytes%256 assert (stride is 256B)."""
    gp = nc.gpsimd
    _in_ap = gp.lower_ap_dma(in_ap, for_custom_bir_dma=True)
    _idxs_ap = gp.lower_ap(idxs_ap)
    _out_ap = gp.lower_ap(out_ap)
    stride_bytes = in_ap.ap[0][0] * mybir.dt.size(in_ap.dtype)
    assert stride_bytes % 256 == 0
    return gp.add_instruction(
        mybir.InstDMAGatherAnt(
            name=nc.get_next_instruction_name(),
            ins=[*_in_ap, _idxs_ap, gp.lower_val_access(gp.to_reg(num_idxs))],
            outs=[_out_ap],
            transpose=False,
            num_idxs=num_idxs,
            elem_size=elem_size,
            stride_bytes_256=stride_bytes // 256,
            gen_mode=0,
            single_packet=single_packet,
            queue_num=queue_num,
            sbuf_tokens_per_rank=0,
            sbuf_free_dim_per_rank=0,
            sbuf_free_dim_pad_per_rank=0,
            sbuf_byte_offset=0,
        )
    )


CHUNK = 32  # gather columns per instruction (32*128 = 4096 idx, ring-safe)


def build(g_list, reps: int = 1, mode: str = "full"):
    """Build + compile the per-core Bass module.

    g_list: [NBLK] uniform per-block slot count (same for all NQ passes).
    The gather destination is one big pass-major tile [P, NQ*SG, D]; each
    pass issues ceil(SG/CHUNK) large gather instructions on its own SWDGE
    queue (issued round-robin across passes so Tile's DMASW lanes stay
    periodic), and each block reduces with a single 4D-AP tensor_reduce.
    reps > 1 wraps the body in tc.For_i for slope timing.
    mode: "full" | "gather" (skip reduce/scale/out).
    """
    g_list = [int(g) for g in g_list]
    assert len(g_list) == NBLK and all(g >= 1 for g in g_list)
    SG = sum(g_list)  # columns per pass
    W = NQ * SG
    WC = W * 8  # idx16 columns (8 per gather column)

    nc = bacc.Bacc("TRN2", target_bir_lowering=False, debug=False,
                   num_swdge_queues=NQ, dynamic_dma_scratch_size=32768)
    table = nc.dram_tensor("table", [DEVROWS, P], mybir.dt.float16,
                           kind="ExternalInput")
    idx = nc.dram_tensor("idx", [P, WC], mybir.dt.int16, kind="ExternalInput")
    inv_len = nc.dram_tensor("inv_len", [P, NBLK], mybir.dt.float32,
                             kind="ExternalInput")
    out = nc.dram_tensor("out", [NBLK, P, D], mybir.dt.float32,
                         kind="ExternalOutput")

    with tile.TileContext(nc) as tc:
        with (
            tc.tile_pool(name="const", bufs=1) as cpool,
            tc.tile_pool(name="res", bufs=4) as rpool,
        ):
            idx_sb = cpool.tile([P, WC], mybir.dt.int16)
            nc.sync.dma_start(idx_sb[:], idx.ap())
            invl_sb = cpool.tile([P, NBLK], mybir.dt.float32)
            nc.sync.dma_start(invl_sb[:], inv_len.ap())
            big = cpool.tile([P, W, D], mybir.dt.float16)

            def body():
                nchunks = -(-SG // CHUNK)
                for j in range(nchunks):
                    c0, c1 = j * CHUNK, min((j + 1) * CHUNK, SG)
                    for q in range(NQ):
                        win = table.ap()[BASES[q] : BASES[q] + WIN, :D]
                        gc0, gc1 = q * SG + c0, q * SG + c1
                        _manual_dma_gather(
                            nc,
                            big[:, gc0:gc1, :],
                            win,
                            idx_sb[:, gc0 * 8 : gc1 * 8],
                            (c1 - c0) * P,
                            D,
                            queue_num=q,
                            single_packet=False,
                        )
                if mode == "gather":
                    return
                off = 0
                for b in range(NBLK):
                    gb = g_list[b]
                    v = big[:].rearrange("p (q c) d -> p d q c", q=NQ)
                    red = rpool.tile([P, D], mybir.dt.float32, tag="red")
                    nc.vector.tensor_reduce(
                        out=red[:],
                        in_=v[:, :, :, off : off + gb],
                        axis=mybir.AxisListType.XY,
                        op=mybir.AluOpType.add,
                    )
                    o = rpool.tile([P, D], mybir.dt.float32, tag="o")
                    nc.scalar.activation(
                        o[:],
                        red[:],
                        mybir.ActivationFunctionType.Copy,
                        scale=invl_sb[:, b : b + 1],
                    )
                    nc.sync.dma_start(out.ap()[b], o[:])
                    off += gb

            if reps == 1:
                body()
            else:
                with tc.For_i(0, reps, 1):
                    body()

    nc.compile()
    return nc


def _dev_table(table):
    """fp16 device table [DEVROWS, 128]: zero sentinel at each window base,
    original row r at device position devpos[r]."""
    t16 = np.asarray(table, dtype=np.float32).astype(np.float16)
    dev = np.zeros((DEVROWS, P), np.float16)
    devpos = np.empty(V, np.int64)
    src = 0
    for pos in range(DEVROWS):
        if pos in BASES:
            continue  # zero sentinel
        dev[pos, :D] = t16[src]
        devpos[src] = pos
        src += 1
    assert src == V
    return dev, devpos


def _balance_passes(devrows_sample):
    """Assign each device-row index to a feasible pass, equalizing per-pass
    counts. Returns list of NQ lists of window-relative indices."""
    groups = [[] for _ in range(NQ)]
    flex = []
    for d in devrows_sample:
        feas = [q for q in range(NQ) if BASES[q] <= d < BASES[q] + WIN]
        if len(feas) == 1:
            groups[feas[0]].append(d - BASES[feas[0]])
        else:
            flex.append((d, feas))
    for d, feas in flex:
        q = min(feas, key=lambda q: len(groups[q]))
        groups[q].append(d - BASES[q])
    return groups


def preprocess(table, indices, lengths):
    """Host prep. Returns (in_maps, g_sched, order) where order[r] is the
    original sample id at global dealt rank r."""
    dev, devpos = _dev_table(table)

    idx_np = np.asarray(indices, dtype=np.int64)  # [B, L]
    lens = np.asarray(lengths).astype(np.int64)  # [B]
    inv_len = (1.0 / np.maximum(lens, 1)).astype(np.float32)

    # per-sample pass groups (window-relative indices)
    sample_groups = []
    cnt = np.zeros((B, NQ), np.int64)
    for s in range(B):
        drows = devpos[idx_np[s, : lens[s]]]
        sample_groups.append(_balance_passes(drows))
        cnt[s] = [len(g) for g in sample_groups[s]]

    # greedy deal: assign samples to the 16 global blocks (1024 each),
    # minimizing each block's uniform slot count max(cnt); rank r ->
    # (block r//1024, core (r%1024)//128, partition r%128)
    cmax = cnt.max(1)
    key = cmax * 64 + lens
    pool = np.argsort(-key, kind="stable")
    bmax = np.zeros(NBLK, np.int64)
    fill = np.zeros(NBLK, np.int64)
    assign = np.empty(B, np.int64)
    for s in pool:
        best, bc = -1, None
        for b in range(NBLK):
            if fill[b] >= 1024:
                continue
            cost = max(bmax[b], cmax[s]) - bmax[b]
            if bc is None or cost < bc:
                best, bc = b, cost
        assign[s] = best
        bmax[best] = max(bmax[best], cmax[s])
        fill[best] += 1
    order = np.concatenate([pool[assign[pool] == b] for b in range(NBLK)])

    g_list = [int(max(1, bmax[b])) for b in range(NBLK)]
    SG = sum(g_list)
    offs = np.cumsum([0] + g_list)

    in_maps = []
    for c in range(NCORES):
        # slot matrix M[p, global gather column] (pass-major), sentinel 0
        M = np.zeros((P, NQ * SG), np.int16)
        invl_dev = np.empty((P, NBLK), np.float32)
        for b in range(NBLK):
            ranks = order[b * 1024 + c * P : b * 1024 + (c + 1) * P]
            invl_dev[:, b] = inv_len[ranks]
            for p, s in enumerate(ranks):
                for q in range(NQ):
                    grp = sample_groups[s][q]
                    M[p, q * SG + offs[b] : q * SG + offs[b] + len(grp)] = grp
        # per gather column j: stream i = j_rel*128 + p; each column wraps to
        # an independent [16, 8] int16 block at idx16 cols [8j, 8j+8)
        X = M.T.reshape(NQ * SG, 8, 16).transpose(1, 2, 0)  # [8, 16, ncol]
        idx16_16 = X.transpose(1, 2, 0).reshape(16, NQ * SG * 8)
        idx16 = np.tile(idx16_16, (8, 1))
        in_maps.append(
            {
                "table": dev,
                "idx": np.ascontiguousarray(idx16),
                "inv_len": np.ascontiguousarray(invl_dev),
            }
        )
    return in_maps, g_list, order


def kernel(table, indices, lengths):
    in_maps, g_list, order = preprocess(table, indices, lengths)
    key = tuple(g_list)
    nc = _CACHE.get(key)
    if nc is None:
        nc = _CACHE[key] = build(g_list)
    res = bass_utils.run_bass_kernel_spmd(nc, in_maps, core_ids=list(range(NCORES)))
    full = np.empty((B, D), np.float32)
    for b in range(NBLK):
        for c in range(NCORES):
            ranks = order[b * 1024 + c * P : b * 1024 + (c + 1) * P]
            full[ranks] = res.results[c]["out"][b]
    return full


# revision 17
# speedup vs baseline: 3.4847x; 1.2500x over previous
"""EmbeddingBag-mean (padded ragged gather + masked mean) on 8 Trainium2 cores.

Strategy (data-parallel over batch):
  - Each of the 8 cores owns B/8 = 2048 samples; the embedding table is
    replicated to every core's HBM as fp16, rows padded to 128 elements
    (256 B stride) so the MoE `dma_gather` ucode (stride in 256 B units,
    int16 indices) can gather single 128 B rows.
  - int16 indices only reach 32768 rows, so the gather runs as 4 passes
    over overlapping 32768-row windows (bases ~22412 apart).  A zero
    sentinel row sits at each window base (relative index 0).  Because the
    windows overlap, each sample distributes its indices among feasible
    passes to equalize its per-pass counts, which keeps the per-block
    per-pass slot maxima near len/4 (instead of len/4 + 3 sigma).
  - Samples are globally length-sorted and dealt to (block, core,
    partition) so each block of 128 partitions holds samples of nearly
    equal length; block b needs G[b][q] gather slots for pass q (max over
    the 8 cores' blocks; one SPMD module).  Pad slots -> sentinel row.
  - Device kernel (per core), per block of 128 samples:
      1. 4x dma_gather (one per pass, 4 SWDGE queues round-robin):
         g[p, off_q + j, :] = window_q[idx16[...], :64]   (128 B descs)
      2. one DVE tensor_reduce over all slot columns (fp16 in, fp32 out)
      3. ACT Copy-with-scale by 1/max(len,1) (per-partition scalar)
      4. DMA the [128, 64] fp32 block out
  - Host un-permutes the global deal and returns [B, 64] fp32.
"""

import numpy as np

try:
    import concourse.bacc as bacc
except ImportError:  # harness containers keep the repo at /opt/trn_rl_repo
    import sys

    sys.path.insert(0, "/opt/trn_rl_repo")
    import concourse.bacc as bacc

import concourse.bass as bass
import concourse.mybir as mybir
import concourse.tile as tile
from concourse import bass_utils

B, L, V, D = 16384, 50, 100000, 64
NCORES = 8
P = 128
BC = B // NCORES  # 2048 samples per core
NBLK = BC // P  # 16 blocks of 128 samples
NQ = 4  # gather passes (overlapping windows)
WIN = 32768  # int16-reachable rows per pass
DEVROWS = V + NQ  # table + one zero sentinel per window

# window bases in device-table row space (sentinel zero row at each base)
_SPACING = -(-(DEVROWS - WIN) // (NQ - 1))  # ceil
BASES = [q * _SPACING for q in range(NQ)]
assert BASES[-1] + WIN >= DEVROWS

_CACHE: dict = {}


def _manual_dma_gather(nc, out_ap, in_ap, idxs_ap, num_idxs, elem_size,
                       queue_num, single_packet):
    """dma_gather without the elem_size_bytes%256 assert (stride is 256B)."""
    gp = nc.gpsimd
    _in_ap = gp.lower_ap_dma(in_ap, for_custom_bir_dma=True)
    _idxs_ap = gp.lower_ap(idxs_ap)
    _out_ap = gp.lower_ap(out_ap)
    stride_bytes = in_ap.ap[0][0] * mybir.dt.size(in_ap.dtype)
    assert stride_bytes % 256 == 0
    return gp.add_instruction(
        mybir.InstDMAGatherAnt(
            name=nc.get_next_instruction_name(),
            ins=[*_in_ap, _idxs_ap, gp.lower_val_access(gp.to_reg(num_idxs))],
            outs=[_out_ap],
            transpose=False,
            num_idxs=num_idxs,
            elem_size=elem_size,
            stride_bytes_256=stride_bytes // 256,
            gen_mode=0,
            single_packet=single_packet,
            queue_num=queue_num,
            sbuf_tokens_per_rank=0,
            sbuf_free_dim_per_rank=0,
            sbuf_free_dim_pad_per_rank=0,
            sbuf_byte_offset=0,
        )
    )


def build(g_sched, reps: int = 1, mode: str = "full", qpat=None):
    """Build + compile the per-core Bass module.

    g_sched: [NBLK][NQ] gather slot counts (>=1 each).
    reps > 1 wraps the block loop in tc.For_i for slope timing.
    mode: "full" | "gather" (skip reduce/scale/out) | "nored" (skip reduce).
    """
    g_sched = [list(r) for r in g_sched]
    assert len(g_sched) == NBLK and all(len(r) == NQ for r in g_sched)
    gtot = [sum(r) for r in g_sched]
    g_max = max(gtot)
    # idx16 column layout: per (block, pass) a run of G*P/16 int16 columns
    wcols = [[g * P // 16 for g in r] for r in g_sched]
    WC = sum(sum(r) for r in wcols)

    nc = bacc.Bacc("TRN2", target_bir_lowering=False, debug=False,
                   num_swdge_queues=NQ)
    table = nc.dram_tensor("table", [DEVROWS, P], mybir.dt.float16,
                           kind="ExternalInput")
    idx = nc.dram_tensor("idx", [P, WC], mybir.dt.int16, kind="ExternalInput")
    inv_len = nc.dram_tensor("inv_len", [P, NBLK], mybir.dt.float32,
                             kind="ExternalInput")
    out = nc.dram_tensor("out", [NBLK, P, D], mybir.dt.float32,
                         kind="ExternalOutput")

    with tile.TileContext(nc) as tc:
        with (
            tc.tile_pool(name="const", bufs=1) as cpool,
            tc.tile_pool(name="gather", bufs=4) as gpool,
            tc.tile_pool(name="res", bufs=4) as rpool,
        ):
            idx_sb = cpool.tile([P, WC], mybir.dt.int16)
            nc.sync.dma_start(idx_sb[:], idx.ap())
            invl_sb = cpool.tile([P, NBLK], mybir.dt.float32)
            nc.sync.dma_start(invl_sb[:], inv_len.ap())

            # one gather per (block, pass).  Queue pattern alternates between
            # blocks so the big outer passes split evenly across queues, while
            # staying periodic in issue order (Tile's DMASW lanes lock to the
            # queue of their first user, so the 8-long pattern must repeat).
            QPAT = qpat or [[0, 1, 2, 3], [2, 3, 0, 1]]

            def body():
                col = 0
                for b in range(NBLK):
                    g = gpool.tile([P, g_max, D], mybir.dt.float16, tag="g")
                    off = 0
                    for q in range(NQ):
                        gq = g_sched[b][q]
                        win = table.ap()[BASES[q] : BASES[q] + WIN, :D]
                        _manual_dma_gather(
                            nc,
                            g[:, off : off + gq, :],
                            win,
                            idx_sb[:, col : col + wcols[b][q]],
                            gq * P,
                            D,
                            queue_num=QPAT[b % 2][q],
                            single_packet=(gq <= 8),
                        )
                        off += gq
                        col += wcols[b][q]
                    if mode == "gather":
                        continue
                    red = rpool.tile([P, D], mybir.dt.float32, tag="red")
                    if mode == "nored":
                        nc.vector.tensor_copy(red[:], g[:, 0, :])
                    else:
                        nc.vector.tensor_reduce(
                            out=red[:],
                            in_=g[:, : gtot[b], :].rearrange("p l d -> p d l"),
                            axis=mybir.AxisListType.X,
                            op=mybir.AluOpType.add,
                        )
                    o = rpool.tile([P, D], mybir.dt.float32, tag="o")
                    nc.scalar.activation(
                        o[:],
                        red[:],
                        mybir.ActivationFunctionType.Copy,
                        scale=invl_sb[:, b : b + 1],
                    )
                    nc.sync.dma_start(out.ap()[b], o[:])

            if reps == 1:
                body()
            else:
                with tc.For_i(0, reps, 1):
                    body()

    nc.compile()
    return nc


def _dev_table(table):
    """fp16 device table [DEVROWS, 128]: zero sentinel at each window base,
    original row r at device position devpos[r]."""
    t16 = np.asarray(table, dtype=np.float32).astype(np.float16)
    dev = np.zeros((DEVROWS, P), np.float16)
    devpos = np.empty(V, np.int64)
    src = 0
    for pos in range(DEVROWS):
        if pos in BASES:
            continue  # zero sentinel
        dev[pos, :D] = t16[src]
        devpos[src] = pos
        src += 1
    assert src == V
    return dev, devpos


def _balance_passes(devrows_sample):
    """Assign each device-row index to a feasible pass, equalizing per-pass
    counts. Returns list of NQ lists of window-relative indices."""
    groups = [[] for _ in range(NQ)]
    flex = []
    for d in devrows_sample:
        feas = [q for q in range(NQ) if BASES[q] <= d < BASES[q] + WIN]
        if len(feas) == 1:
            groups[feas[0]].append(d - BASES[feas[0]])
        else:
            flex.append((d, feas))
    for d, feas in flex:
        q = min(feas, key=lambda q: len(groups[q]))
        groups[q].append(d - BASES[q])
    return groups


def preprocess(table, indices, lengths):
    """Host prep. Returns (in_maps, g_sched, order) where order[r] is the
    original sample id at global dealt rank r."""
    dev, devpos = _dev_table(table)

    idx_np = np.asarray(indices, dtype=np.int64)  # [B, L]
    lens = np.asarray(lengths).astype(np.int64)  # [B]
    inv_len = (1.0 / np.maximum(lens, 1)).astype(np.float32)

    # per-sample pass groups (window-relative indices)
    sample_groups = []
    cnt = np.zeros((B, NQ), np.int64)
    for s in range(B):
        drows = devpos[idx_np[s, : lens[s]]]
        sample_groups.append(_balance_passes(drows))
        cnt[s] = [len(g) for g in sample_groups[s]]

    # greedy deal: assign samples to the 16 global blocks (1024 each) to
    # minimize the per-block per-pass maxima; rank r -> (block r//1024,
    # core (r%1024)//128, partition r%128)
    key = cnt.max(1) * 64 + lens
    pool = np.argsort(-key, kind="stable")
    gmax = np.zeros((NBLK, NQ), np.int64)
    fill = np.zeros(NBLK, np.int64)
    assign = np.empty(B, np.int64)
    for s in pool:
        best, bc = -1, None
        for b in range(NBLK):
            if fill[b] >= 1024:
                continue
            cost = np.maximum(gmax[b], cnt[s]).sum() - gmax[b].sum()
            if bc is None or cost < bc:
                best, bc = b, cost
        assign[s] = best
        gmax[best] = np.maximum(gmax[best], cnt[s])
        fill[best] += 1
    order = np.concatenate([pool[assign[pool] == b] for b in range(NBLK)])

    g_sched = [[int(x) for x in np.maximum(gmax[b], 1)] for b in range(NBLK)]

    wcols = [[g * P // 16 for g in r] for r in g_sched]
    WC = sum(sum(r) for r in wcols)

    in_maps = []
    for c in range(NCORES):
        idx16 = np.zeros((P, WC), np.int16)
        invl_dev = np.empty((P, NBLK), np.float32)
        col = 0
        for b in range(NBLK):
            ranks = order[b * 1024 + c * P : b * 1024 + (c + 1) * P]
            invl_dev[:, b] = inv_len[ranks]
            for q in range(NQ):
                gq = g_sched[b][q]
                blk = np.zeros((P, gq), np.int16)  # sentinel rel idx 0
                for p, s in enumerate(ranks):
                    grp = sample_groups[s][q]
                    blk[p, : len(grp)] = grp
                # stream order i = c*128 + p -> wrap int16 [16, nidx/16] x8
                flat = blk.T.ravel()  # [gq*128]
                w = flat.reshape(gq * P // 16, 16).T  # [16, cols]
                nw = wcols[b][q]
                idx16[:, col : col + nw] = np.tile(w, (8, 1))
                col += nw
        in_maps.append(
            {
                "table": dev,
                "idx": np.ascontiguousarray(idx16),
                "inv_len": np.ascontiguousarray(invl_dev),
            }
        )
    return in_maps, g_sched, order


def kernel(table, indices, lengths):
    in_maps, g_sched, order = preprocess(table, indices, lengths)
    key = tuple(tuple(r) for r in g_sched)
    nc = _CACHE.get(key)
    if nc is None:
        nc = _CACHE[key] = build(g_sched)
    res = bass_utils.run_bass_kernel_spmd(nc, in_maps, core_ids=list(range(NCORES)))
    full = np.empty((B, D), np.float32)
    for b in range(NBLK):
        for c in range(NCORES):
            ranks = order[b * 1024 + c * P : b * 1024 + (c + 1) * P]
            full[ranks] = res.results[c]["out"][b]
    return full
